# revision 4
# baseline (speedup 1.0000x reference)
"""Trainium2 Bass kernel for nn_Attention_51470888075468.

Spatial-reduction attention block (PVT-style) over B=32, N=1280, C=256,
8 heads (hd=32). Search image 32x32 (1024 tokens) + template 16x16 (256
tokens); k/v come from a stride-2 2x2 conv (-> M=320 kv tokens) + LayerNorm.

Sharding: pure data-parallel over batch. 8 NeuronCores x 4 batches each.
Weights replicated. No collectives.

On-chip dataflow (per core, per batch), all feature-major [C, tokens]:
  x -> (cast bf16, xbar-transpose) -> xT [C, 1280]
  qT = qW^T-stationary matmuls            [C, 1280]
  y  = conv via 4 strided accumulating matmuls + bias      [C, 320]
  LN stats via ones-matmul column sums; rsqrt as exp(-0.5*ln(var+eps))
  y_n broadcast-normalized (K=1 ones matmuls for mu/r broadcast)
  kT = kvW[:256]-stationary  [C, 320];  v token-major via swapped operands
  scores s^T[m, n-chunk] with 4 heads row-packed (K=32, tile_position)
  p = exp(scale*s) on ScalarE, PSUM->SBUF bf16
  column sums + attnV col-packed 4-heads-wide, accumulated over m-tiles
  out_n = attnV * 1/colsum ; proj emits token-major -> contiguous store
"""

import sys

for _p in ("/opt/trn_rl_repo",):
    if _p not in sys.path:
        sys.path.insert(0, _p)

from contextlib import ExitStack

import numpy as np

import concourse.bass as bass
import concourse.tile as tile
from concourse import bacc, mybir
from concourse.bass_utils import run_bass_kernel_spmd
from concourse.masks import make_identity

F32 = mybir.dt.float32
BF16 = mybir.dt.bfloat16

NCORES = 8
B_LOC = 4          # batches per core
N = 1280           # query tokens
C = 256            # channels
H = 8              # heads
HD = 32            # head dim
M = 320            # kv tokens after sr-conv (16*16 + 8*8)
SCALE = float(HD) ** -0.5
LN_EPS = 1e-5
# attention n-chunks
CHUNKS = [(0, 512), (512, 512), (1024, 256)]
# m-tiles of the 320 kv tokens
MTILES = [(0, 128), (128, 128), (256, 64)]


def build_kernel():
    nc = bacc.Bacc("TRN2", target_bir_lowering=False, debug=False,
                   num_devices=NCORES)

    x_d = nc.dram_tensor("x", [B_LOC, N, C], F32, kind="ExternalInput")
    qw_d = nc.dram_tensor("q_w", [C, C], F32, kind="ExternalInput")
    kvw_d = nc.dram_tensor("kv_w", [2 * C, C], F32, kind="ExternalInput")
    srw_d = nc.dram_tensor("sr_w", [C, C, 2, 2], F32, kind="ExternalInput")
    srb_d = nc.dram_tensor("sr_b", [C], F32, kind="ExternalInput")
    lng_d = nc.dram_tensor("ln_g", [C], F32, kind="ExternalInput")
    lnb_d = nc.dram_tensor("ln_b", [C], F32, kind="ExternalInput")
    pw_d = nc.dram_tensor("proj_w", [C, C], F32, kind="ExternalInput")
    pb_d = nc.dram_tensor("proj_b", [C], F32, kind="ExternalInput")
    out_d = nc.dram_tensor("out", [B_LOC, N, C], F32, kind="ExternalOutput")

    with tile.TileContext(nc) as tc, ExitStack() as ctx:
        build_body(tc, ctx, x_d, qw_d, kvw_d, srw_d, srb_d, lng_d, lnb_d,
                   pw_d, pb_d, out_d)
    nc.compile()
    return nc


def build_body(tc, ctx, x_d, qw_d, kvw_d, srw_d, srb_d, lng_d, lnb_d,
               pw_d, pb_d, out_d):
    nc = tc.nc
    x = x_d.ap()
    out = out_d.ap()

    # ---------------- pools ----------------
    consts = ctx.enter_context(tc.tile_pool(name="consts", bufs=1))
    wstage = ctx.enter_context(tc.tile_pool(name="wstage", bufs=1))
    # PSUM: s_pool slot [128,2048]f32 = 4 banks (bufs=1) + ro_pool slot
    # [128,1024]f32 = 2 banks (bufs=2) -> 8 banks total.
    s_pool = ctx.enter_context(tc.tile_pool(name="s_psum", bufs=1, space="PSUM"))
    ro_pool = ctx.enter_context(tc.tile_pool(name="ro_psum", bufs=2, space="PSUM"))
    xq_pool = ctx.enter_context(tc.tile_pool(name="xq", bufs=2))
    mid_pool = ctx.enter_context(tc.tile_pool(name="mid", bufs=2))
    p_pool = ctx.enter_context(tc.tile_pool(name="pexp", bufs=2))
    o_pool = ctx.enter_context(tc.tile_pool(name="osb", bufs=3))
    dram_pool = ctx.enter_context(tc.tile_pool(name="dstage", bufs=2, space="DRAM"))

    # ---------------- constants ----------------
    ident = consts.tile([128, 128], F32, tag="ident")
    make_identity(nc, ident)
    ones_col = consts.tile([128, 1], F32, tag="ones_col")   # LN stats lhsT
    nc.vector.memset(ones_col, 1.0)
    ones_row = consts.tile([1, 128], F32, tag="ones_row")   # K=1 bcast lhsT
    nc.vector.memset(ones_row, 1.0)
    ones_bf = consts.tile([128, 32], BF16, tag="ones_bf")   # colsum lhsT
    nc.vector.memset(ones_bf, 1.0)
    zeros_b = consts.tile([128, 1], F32, tag="zeros_b")     # act bias = 0
    nc.vector.memset(zeros_b, 0.0)
    eps_b = consts.tile([1, 1], F32, tag="eps_b")           # act bias = eps
    nc.vector.memset(eps_b, LN_EPS)

    # per-partition vectors [128,1] x 2 ctiles
    def load_vec(name, dram):
        tiles = []
        for t in range(2):
            v = consts.tile([128, 1], F32, tag=f"{name}{t}", name=f"{name}{t}")
            nc.sync.dma_start(out=v, in_=dram.ap()[t * 128:(t + 1) * 128][:, None])
            tiles.append(v)
        return tiles

    srb_sb = load_vec("srb", srb_d)
    g_sb = load_vec("lng", lng_d)
    b_sb = load_vec("lnb", lnb_d)

    # proj bias broadcast to [128, 256] via partition-step-0 DMA
    pb_bc = consts.tile([128, C], F32, tag="pb_bc")
    pb_ap = bass.AP(tensor=pb_d, offset=0, ap=[[0, 128], [1, C]])
    nc.gpsimd.dma_start(out=pb_bc, in_=pb_ap)

    # ---------------- weights: load natural f32, PE-transpose, cast bf16 ----
    qw_nat = []
    kvw_nat = []
    pw_nat = []
    srw_nat = []
    for t in range(2):
        w = wstage.tile([128, C], F32, tag=f"qwn{t}", name=f"qwn{t}")
        nc.sync.dma_start(out=w, in_=qw_d.ap()[t * 128:(t + 1) * 128, :])
        qw_nat.append(w)
    for t in range(4):
        w = wstage.tile([128, C], F32, tag=f"kvwn{t}", name=f"kvwn{t}")
        nc.sync.dma_start(out=w, in_=kvw_d.ap()[t * 128:(t + 1) * 128, :])
        kvw_nat.append(w)
    for t in range(2):
        w = wstage.tile([128, C], F32, tag=f"pwn{t}", name=f"pwn{t}")
        nc.sync.dma_start(out=w, in_=pw_d.ap()[t * 128:(t + 1) * 128, :])
        pw_nat.append(w)
    srw_r = srw_d.ap().rearrange("o i kh kw -> o (i kh kw)")
    for t in range(2):
        w = wstage.tile([128, C * 4], F32, tag=f"srwn{t}", name=f"srwn{t}")
        nc.sync.dma_start(out=w, in_=srw_r[t * 128:(t + 1) * 128, :])
        srw_nat.append(w)

    qwT = [consts.tile([128, C], BF16, tag=f"qwT{t}", name=f"qwT{t}") for t in range(2)]
    kvwT = [consts.tile([128, 2 * C], BF16, tag=f"kvwT{t}", name=f"kvwT{t}") for t in range(2)]
    pwT = [consts.tile([128, C], BF16, tag=f"pwT{t}", name=f"pwT{t}") for t in range(2)]
    # srwT[ci_t][kh*2+kw] : [128(i), 256(o)]
    srwT = [[consts.tile([128, C], BF16, tag=f"srwT{t}_{k}", name=f"srwT{t}_{k}") for k in range(4)]
            for t in range(2)]

    def pe_transpose(dst_bf16_slice, src_f32_128x128):
        ps = ro_pool.tile([128, 1024], F32, tag="ro")
        nc.tensor.transpose(ps[:, 0:128], src_f32_128x128, ident)
        nc.vector.tensor_copy(dst_bf16_slice, ps[:, 0:128])

    for ci in range(2):
        for co in range(2):
            pe_transpose(qwT[ci][:, co * 128:(co + 1) * 128],
                         qw_nat[co][:, ci * 128:(ci + 1) * 128])
            pe_transpose(pwT[ci][:, co * 128:(co + 1) * 128],
                         pw_nat[co][:, ci * 128:(ci + 1) * 128])
        for ko in range(4):
            pe_transpose(kvwT[ci][:, ko * 128:(ko + 1) * 128],
                         kvw_nat[ko][:, ci * 128:(ci + 1) * 128])
        for k in range(4):
            for ot in range(2):
                src = srw_nat[ot].rearrange("p (i k) -> p i k", k=4)
                pe_transpose(srwT[ci][k][:, ot * 128:(ot + 1) * 128],
                             src[:, ci * 128:(ci + 1) * 128, k])

    # ---------------- per-batch ----------------
    for b in range(B_LOC):
        # cast x[b] f32 -> bf16 in DRAM, then xbar-transpose to [C, N]
        xbf = dram_pool.tile([N, C], BF16, tag="xbf")
        nc.gpsimd.dma_start(out=xbf, in_=x[b])
        xT = [xq_pool.tile([128, N], BF16, tag=f"xT{t}", name=f"xT{t}") for t in range(2)]
        for t in range(2):
            nc.sync.dma_start_transpose(out=xT[t],
                                        in_=xbf[:, t * 128:(t + 1) * 128])

        # ---- qT[co_t] = q_w @ x^T  (feature-major) ----
        qT = [xq_pool.tile([128, N], BF16, tag=f"qT{t}", name=f"qT{t}") for t in range(2)]
        for cot in range(2):
            for (c0, cw) in CHUNKS:
                ps = ro_pool.tile([128, 1024], F32, tag="ro")
                for cit in range(2):
                    nc.tensor.matmul(ps[:, 0:cw],
                                     lhsT=qwT[cit][:, cot * 128:(cot + 1) * 128],
                                     rhs=xT[cit][:, c0:c0 + cw],
                                     start=(cit == 0), stop=(cit == 1))
                nc.vector.tensor_copy(qT[cot][:, c0:c0 + cw], ps[:, 0:cw])

        # ---- conv y[o_t] [128, 320] + bias ----
        ybuf = [mid_pool.tile([128, M], F32, tag=f"ybuf{t}", name=f"ybuf{t}") for t in range(2)]
        for ot in range(2):
            ps = ro_pool.tile([128, 1024], F32, tag="ro")
            # search: 32x32 -> 16x16 (tokens 0:1024)
            xs = [xT[t][:, 0:1024].rearrange("p (r a c b) -> p r a c b",
                                             r=16, a=2, c=16, b=2)
                  for t in range(2)]
            first = True
            for cit in range(2):
                for kh in range(2):
                    for kw in range(2):
                        nc.tensor.matmul(
                            ps[:, 0:256],
                            lhsT=srwT[cit][kh * 2 + kw][:, ot * 128:(ot + 1) * 128],
                            rhs=xs[cit][:, :, kh, :, kw],
                            start=first, stop=(cit == 1 and kh == 1 and kw == 1))
                        first = False
            # template: 16x16 -> 8x8 (tokens 1024:1280)
            xt_ = [xT[t][:, 1024:1280].rearrange("p (r a c b) -> p r a c b",
                                                 r=8, a=2, c=8, b=2)
                   for t in range(2)]
            first = True
            for cit in range(2):
                for kh in range(2):
                    for kw in range(2):
                        nc.tensor.matmul(
                            ps[:, 512:576],
                            lhsT=srwT[cit][kh * 2 + kw][:, ot * 128:(ot + 1) * 128],
                            rhs=xt_[cit][:, :, kh, :, kw],
                            start=first, stop=(cit == 1 and kh == 1 and kw == 1))
                        first = False
            nc.vector.tensor_scalar_add(ybuf[ot][:, 0:256], ps[:, 0:256],
                                        srb_sb[ot])
            nc.vector.tensor_scalar_add(ybuf[ot][:, 256:320], ps[:, 512:576],
                                        srb_sb[ot])

        # ---- LN stats (over channel = partitions, via ones-matmuls) ----
        ysq = [mid_pool.tile([128, M], F32, tag=f"ysq{t}", name=f"ysq{t}") for t in range(2)]
        for ot in range(2):
            nc.vector.tensor_mul(ysq[ot], ybuf[ot], ybuf[ot])
        ps_stat = ro_pool.tile([128, 1024], F32, tag="ro")
        for ot in range(2):
            nc.tensor.matmul(ps_stat[0:1, 0:M], lhsT=ones_col, rhs=ybuf[ot],
                             start=(ot == 0), stop=(ot == 1))
        for ot in range(2):
            nc.tensor.matmul(ps_stat[0:1, 512:512 + M], lhsT=ones_col,
                             rhs=ysq[ot], start=(ot == 0), stop=(ot == 1))
        mu = mid_pool.tile([1, M], F32, tag="mu")
        nc.vector.tensor_scalar_mul(mu, ps_stat[0:1, 0:M], 1.0 / C)
        var = mid_pool.tile([1, M], F32, tag="var")
        # var = E[y^2] - mu^2
        musq = mid_pool.tile([1, M], F32, tag="musq")
        nc.vector.tensor_mul(musq, mu, mu)
        nc.vector.tensor_scalar_mul(var, ps_stat[0:1, 512:512 + M], 1.0 / C)
        nc.vector.tensor_sub(var, var, musq)
        # r = rsqrt(var+eps) = exp(-0.5*ln(var+eps))
        lnv = mid_pool.tile([1, M], F32, tag="lnv")
        nc.scalar.activation(lnv, var, mybir.ActivationFunctionType.Ln,
                             bias=eps_b, scale=1.0)
        rstd = mid_pool.tile([1, M], F32, tag="rstd")
        nc.scalar.activation(rstd, lnv, mybir.ActivationFunctionType.Exp,
                             bias=zeros_b[0:1], scale=-0.5)
        # broadcast mu, r across partitions via K=1 matmul
        ps_bc = ro_pool.tile([128, 1024], F32, tag="ro")
        nc.tensor.matmul(ps_bc[:, 0:M], lhsT=ones_row, rhs=mu,
                         start=True, stop=True)
        nc.tensor.matmul(ps_bc[:, 512:512 + M], lhsT=ones_row, rhs=rstd,
                         start=True, stop=True)

        # ---- y_n = (y - mu_b) * (r_b * g) + b   (bf16) ----
        y_n = [mid_pool.tile([128, M], BF16, tag=f"yn{t}", name=f"yn{t}") for t in range(2)]
        for ot in range(2):
            t1 = mid_pool.tile([128, M], F32, tag=f"t1_{ot}", name=f"t1_{ot}")
            nc.vector.tensor_sub(t1, ybuf[ot], ps_bc[:, 0:M])
            rg = mid_pool.tile([128, M], F32, tag=f"rg_{ot}", name=f"rg_{ot}")
            nc.vector.tensor_scalar_mul(rg, ps_bc[:, 512:512 + M], g_sb[ot])
            t2 = mid_pool.tile([128, M], F32, tag=f"t2_{ot}", name=f"t2_{ot}")
            nc.vector.tensor_mul(t2, t1, rg)
            nc.vector.tensor_scalar_add(y_n[ot], t2, b_sb[ot])

        # ---- kT (feature-major) and v (token-major) ----
        kT = [mid_pool.tile([128, M], BF16, tag=f"kT{t}", name=f"kT{t}") for t in range(2)]
        for kot in range(2):
            ps = ro_pool.tile([128, 1024], F32, tag="ro")
            for cit in range(2):
                nc.tensor.matmul(ps[:, 0:M],
                                 lhsT=kvwT[cit][:, kot * 128:(kot + 1) * 128],
                                 rhs=y_n[cit],
                                 start=(cit == 0), stop=(cit == 1))
            nc.vector.tensor_copy(kT[kot], ps[:, 0:M])
        v_sb = [mid_pool.tile([128, C], BF16, tag=f"v{i}", name=f"v{i}") for i in range(3)]
        for mi, (m0, mw) in enumerate(MTILES):
            ps = ro_pool.tile([128, 1024], F32, tag="ro")
            for cit in range(2):
                nc.tensor.matmul(ps[0:mw, 0:C],
                                 lhsT=y_n[cit][:, m0:m0 + mw],
                                 rhs=kvwT[cit][:, C:2 * C],
                                 start=(cit == 0), stop=(cit == 1))
            nc.vector.tensor_copy(v_sb[mi][0:mw, :], ps[0:mw, 0:C])

        # ---- attention ----
        for (c0, cw) in CHUNKS:
            # accumulators: [:, 0:cw] = attnV, [:, 512:512+cw] = colsum
            acc = [ro_pool.tile([128, 1024], F32, tag="ro", name="acc") for _ in range(2)]
            for mi, (m0, mw) in enumerate(MTILES):
                for g in range(2):
                    s_ps = s_pool.tile([128, 4 * 512], F32, tag="s")
                    for hh in range(4):
                        nc.tensor.matmul(
                            s_ps[0:mw, hh * 512:hh * 512 + cw],
                            lhsT=kT[g][hh * 32:(hh + 1) * 32, m0:m0 + mw],
                            rhs=qT[g][hh * 32:(hh + 1) * 32, c0:c0 + cw],
                            start=True, stop=True,
                            tile_position=(32 * hh, 0))
                    p_sb = p_pool.tile([128, 4 * 512], BF16, tag="p")
                    s_v = s_ps.rearrange("p (h w) -> p h w", h=4)[0:mw, :, 0:cw]
                    p_v = p_sb.rearrange("p (h w) -> p h w", h=4)[0:mw, :, 0:cw]
                    nc.scalar.activation(p_v, s_v,
                                         mybir.ActivationFunctionType.Exp,
                                         bias=zeros_b[0:mw], scale=SCALE)
                    for hh in range(4):
                        nc.tensor.matmul(
                            acc[g][32 * hh:32 * (hh + 1), 512:512 + cw],
                            lhsT=ones_bf[0:mw, :],
                            rhs=p_sb[0:mw, hh * 512:hh * 512 + cw],
                            start=(mi == 0), stop=(mi == 2),
                            tile_position=(0, 32 * hh),
                            skip_group_check=(hh > 0))
                        nc.tensor.matmul(
                            acc[g][32 * hh:32 * (hh + 1), 0:cw],
                            lhsT=v_sb[mi][0:mw,
                                          g * 128 + hh * 32:g * 128 + (hh + 1) * 32],
                            rhs=p_sb[0:mw, hh * 512:hh * 512 + cw],
                            start=(mi == 0), stop=(mi == 2),
                            tile_position=(0, 32 * hh),
                            skip_group_check=(hh > 0))
            outT = []
            for g in range(2):
                rrec = o_pool.tile([128, 512], F32, tag=f"rrec{g}", name=f"rrec{g}")
                nc.vector.reciprocal(rrec[:, 0:cw], acc[g][:, 512:512 + cw])
                on = o_pool.tile([128, 512], BF16, tag=f"outT{g}", name=f"outT{g}")
                nc.vector.tensor_mul(on[:, 0:cw], acc[g][:, 0:cw], rrec[:, 0:cw])
                outT.append(on)
            # proj: token-major output tiles
            for nt in range(cw // 128):
                ps = ro_pool.tile([128, 1024], F32, tag="ro")
                for ct in range(2):
                    nc.tensor.matmul(ps[:, 0:C],
                                     lhsT=outT[ct][:, nt * 128:(nt + 1) * 128],
                                     rhs=pwT[ct],
                                     start=(ct == 0), stop=(ct == 1))
                ob = o_pool.tile([128, C], F32, tag="ob")
                nc.vector.tensor_add(ob, ps[:, 0:C], pb_bc)
                n0 = c0 + nt * 128
                nc.sync.dma_start(out=out[b, n0:n0 + 128, :], in_=ob)


_NC_CACHE = None


def _get_nc():
    global _NC_CACHE
    if _NC_CACHE is None:
        _NC_CACHE = build_kernel()
    return _NC_CACHE


def kernel(**inputs) -> np.ndarray:
    x = np.ascontiguousarray(np.asarray(inputs["x"], dtype=np.float32))
    B = x.shape[0]
    assert x.shape == (32, N, C), x.shape
    weights = {k: np.ascontiguousarray(np.asarray(inputs[k], dtype=np.float32))
               for k in ("q_w", "kv_w", "sr_w", "sr_b", "ln_g", "ln_b",
                         "proj_w", "proj_b")}
    nc = _get_nc()
    in_maps = []
    for core in range(NCORES):
        m = {"x": x[core * B_LOC:(core + 1) * B_LOC]}
        m.update(weights)
        in_maps.append(m)
    res = run_bass_kernel_spmd(nc, in_maps, core_ids=list(range(NCORES)))
    out = np.concatenate([res.results[i]["out"] for i in range(NCORES)], axis=0)
    assert out.shape == (B, N, C)
    return out.astype(np.float32)


if __name__ == "__main__":
    rng = np.random.default_rng(0)
    ins = {
        "x": rng.standard_normal((32, N, C), dtype=np.float32),
        "q_w": rng.standard_normal((C, C), dtype=np.float32) * 0.02,
        "kv_w": rng.standard_normal((2 * C, C), dtype=np.float32) * 0.02,
        "sr_w": rng.standard_normal((C, C, 2, 2), dtype=np.float32) * 0.02,
        "sr_b": np.zeros(C, np.float32),
        "ln_g": np.ones(C, np.float32),
        "ln_b": np.zeros(C, np.float32),
        "proj_w": rng.standard_normal((C, C), dtype=np.float32) * 0.02,
        "proj_b": np.zeros(C, np.float32),
    }
    o = kernel(**ins)
    print("out", o.shape, o.dtype, float(np.abs(o).max()))


# revision 7
# speedup vs baseline: 1.2364x; 1.2364x over previous
"""Trainium2 Bass kernel for nn_Attention_51470888075468.

Spatial-reduction attention block (PVT-style) over B=32, N=1280, C=256,
8 heads (hd=32). Search image 32x32 (1024 tokens) + template 16x16 (256
tokens); k/v come from a stride-2 2x2 conv (-> M=320 kv tokens) + LayerNorm.

Sharding: pure data-parallel over batch. 8 NeuronCores x 4 batches each.
Weights replicated. No collectives.

On-chip dataflow (per core, per batch), all feature-major [C, tokens]:
  x -> (cast bf16, xbar-transpose) -> xT [C, 1280]
  qT = qW^T-stationary matmuls            [C, 1280]
  y  = conv via 4 strided accumulating matmuls + bias      [C, 320]
  LN stats via ones-matmul column sums; rsqrt as exp(-0.5*ln(var+eps))
  y_n broadcast-normalized (K=1 ones matmuls for mu/r broadcast)
  kT = kvW[:256]-stationary  [C, 320];  v token-major via swapped operands
  scores s^T[m, n-chunk] with 4 heads row-packed (K=32, tile_position)
  p = exp(scale*s) on ScalarE, PSUM->SBUF bf16
  column sums + attnV col-packed 4-heads-wide, accumulated over m-tiles
  out_n = attnV * 1/colsum ; proj emits token-major -> contiguous store
"""

import sys

for _p in ("/opt/trn_rl_repo",):
    if _p not in sys.path:
        sys.path.insert(0, _p)

from contextlib import ExitStack

import numpy as np

import concourse.bass as bass
import concourse.tile as tile
from concourse import bacc, mybir
from concourse.bass_utils import run_bass_kernel_spmd
from concourse.masks import make_identity

F32 = mybir.dt.float32
BF16 = mybir.dt.bfloat16

NCORES = 8
B_LOC = 4          # batches per core
N = 1280           # query tokens
C = 256            # channels
H = 8              # heads
HD = 32            # head dim
M = 320            # kv tokens after sr-conv (16*16 + 8*8)
SCALE = float(HD) ** -0.5
LN_EPS = 1e-5
# attention n-chunks
CHUNKS = [(0, 512), (512, 512), (1024, 256)]
# m-tiles of the 320 kv tokens
MTILES = [(0, 128), (128, 128), (256, 64)]


def build_kernel():
    nc = bacc.Bacc("TRN2", target_bir_lowering=False, debug=False,
                   num_devices=NCORES)

    x_d = nc.dram_tensor("x", [B_LOC, N, C], F32, kind="ExternalInput")
    qw_d = nc.dram_tensor("q_w", [C, C], F32, kind="ExternalInput")
    kvw_d = nc.dram_tensor("kv_w", [2 * C, C], F32, kind="ExternalInput")
    srw_d = nc.dram_tensor("sr_w", [C, C, 2, 2], F32, kind="ExternalInput")
    srb_d = nc.dram_tensor("sr_b", [C], F32, kind="ExternalInput")
    lng_d = nc.dram_tensor("ln_g", [C], F32, kind="ExternalInput")
    lnb_d = nc.dram_tensor("ln_b", [C], F32, kind="ExternalInput")
    pw_d = nc.dram_tensor("proj_w", [C, C], F32, kind="ExternalInput")
    pb_d = nc.dram_tensor("proj_b", [C], F32, kind="ExternalInput")
    out_d = nc.dram_tensor("out", [B_LOC, N, C], F32, kind="ExternalOutput")

    with tile.TileContext(nc) as tc, ExitStack() as ctx:
        build_body(tc, ctx, x_d, qw_d, kvw_d, srw_d, srb_d, lng_d, lnb_d,
                   pw_d, pb_d, out_d)
    nc.compile()
    return nc


def build_body(tc, ctx, x_d, qw_d, kvw_d, srw_d, srb_d, lng_d, lnb_d,
               pw_d, pb_d, out_d):
    nc = tc.nc
    x = x_d.ap()
    out = out_d.ap()

    # ---------------- pools ----------------
    consts = ctx.enter_context(tc.tile_pool(name="consts", bufs=1))
    wstage = ctx.enter_context(tc.tile_pool(name="wstage", bufs=1))
    # PSUM: s_pool slot [128,2048]f32 = 4 banks (bufs=1) + ro_pool slot
    # [128,1024]f32 = 2 banks (bufs=2) -> 8 banks total.
    s_pool = ctx.enter_context(tc.tile_pool(name="s_psum", bufs=1, space="PSUM"))
    ro_pool = ctx.enter_context(tc.tile_pool(name="ro_psum", bufs=2, space="PSUM"))
    xq_pool = ctx.enter_context(tc.tile_pool(name="xq", bufs=2))
    mid_pool = ctx.enter_context(tc.tile_pool(name="mid", bufs=2))
    p_pool = ctx.enter_context(tc.tile_pool(name="pexp", bufs=2))
    o_pool = ctx.enter_context(tc.tile_pool(name="osb", bufs=3))
    dram_pool = ctx.enter_context(tc.tile_pool(name="dstage", bufs=2, space="DRAM"))

    # ---------------- constants ----------------
    ident = consts.tile([128, 128], F32, tag="ident")
    make_identity(nc, ident)
    ones_col = consts.tile([128, 1], F32, tag="ones_col")   # LN stats lhsT
    nc.vector.memset(ones_col, 1.0)
    ones_row = consts.tile([1, 128], F32, tag="ones_row")   # K=1 bcast lhsT
    nc.vector.memset(ones_row, 1.0)
    ones_bf = consts.tile([128, 32], BF16, tag="ones_bf")   # colsum lhsT
    nc.vector.memset(ones_bf, 1.0)
    zeros_b = consts.tile([128, 1], F32, tag="zeros_b")     # act bias = 0
    nc.vector.memset(zeros_b, 0.0)
    eps_b = consts.tile([1, 1], F32, tag="eps_b")           # act bias = eps
    nc.vector.memset(eps_b, LN_EPS)

    # per-partition vectors [128,1] x 2 ctiles
    def load_vec(name, dram):
        tiles = []
        for t in range(2):
            v = consts.tile([128, 1], F32, tag=f"{name}{t}", name=f"{name}{t}")
            nc.sync.dma_start(out=v, in_=dram.ap()[t * 128:(t + 1) * 128][:, None])
            tiles.append(v)
        return tiles

    srb_sb = load_vec("srb", srb_d)
    g_sb = load_vec("lng", lng_d)
    b_sb = load_vec("lnb", lnb_d)

    # proj bias broadcast to [128, 256] via partition-step-0 DMA
    pb_bc = consts.tile([128, C], F32, tag="pb_bc")
    pb_ap = bass.AP(tensor=pb_d, offset=0, ap=[[0, 128], [1, C]])
    nc.gpsimd.dma_start(out=pb_bc, in_=pb_ap)

    # ---------------- weights: load natural f32, PE-transpose, cast bf16 ----
    qw_nat = []
    kvw_nat = []
    pw_nat = []
    srw_nat = []
    for t in range(2):
        w = wstage.tile([128, C], F32, tag=f"qwn{t}", name=f"qwn{t}")
        nc.sync.dma_start(out=w, in_=qw_d.ap()[t * 128:(t + 1) * 128, :])
        qw_nat.append(w)
    for t in range(4):
        w = wstage.tile([128, C], F32, tag=f"kvwn{t}", name=f"kvwn{t}")
        nc.sync.dma_start(out=w, in_=kvw_d.ap()[t * 128:(t + 1) * 128, :])
        kvw_nat.append(w)
    for t in range(2):
        w = wstage.tile([128, C], F32, tag=f"pwn{t}", name=f"pwn{t}")
        nc.sync.dma_start(out=w, in_=pw_d.ap()[t * 128:(t + 1) * 128, :])
        pw_nat.append(w)
    srw_r = srw_d.ap().rearrange("o i kh kw -> o (i kh kw)")
    for t in range(2):
        w = wstage.tile([128, C * 4], F32, tag=f"srwn{t}", name=f"srwn{t}")
        nc.sync.dma_start(out=w, in_=srw_r[t * 128:(t + 1) * 128, :])
        srw_nat.append(w)

    qwT = [consts.tile([128, C], BF16, tag=f"qwT{t}", name=f"qwT{t}") for t in range(2)]
    kvwT = [consts.tile([128, 2 * C], BF16, tag=f"kvwT{t}", name=f"kvwT{t}") for t in range(2)]
    pwT = [consts.tile([128, C], BF16, tag=f"pwT{t}", name=f"pwT{t}") for t in range(2)]
    # srwT[ci_t][kh*2+kw] : [128(i), 256(o)]
    srwT = [[consts.tile([128, C], BF16, tag=f"srwT{t}_{k}", name=f"srwT{t}_{k}") for k in range(4)]
            for t in range(2)]

    def pe_transpose(dst_bf16_slice, src_f32_128x128):
        ps = ro_pool.tile([128, 1024], F32, tag="ro")
        nc.tensor.transpose(ps[:, 0:128], src_f32_128x128, ident)
        nc.vector.tensor_copy(dst_bf16_slice, ps[:, 0:128])

    for ci in range(2):
        for co in range(2):
            pe_transpose(qwT[ci][:, co * 128:(co + 1) * 128],
                         qw_nat[co][:, ci * 128:(ci + 1) * 128])
            pe_transpose(pwT[ci][:, co * 128:(co + 1) * 128],
                         pw_nat[co][:, ci * 128:(ci + 1) * 128])
        for ko in range(4):
            pe_transpose(kvwT[ci][:, ko * 128:(ko + 1) * 128],
                         kvw_nat[ko][:, ci * 128:(ci + 1) * 128])
        for k in range(4):
            for ot in range(2):
                src = srw_nat[ot].rearrange("p (i k) -> p i k", k=4)
                pe_transpose(srwT[ci][k][:, ot * 128:(ot + 1) * 128],
                             src[:, ci * 128:(ci + 1) * 128, k])

    # ------- phase 1: x load/transpose + conv + LN stats (all batches) -------
    xT_all = []
    ybuf_all = []
    mu_all = []
    rstd_all = []
    for b in range(B_LOC):
        xbf = dram_pool.tile([N, C], BF16, tag="xbf", name="xbf", bufs=2)
        nc.gpsimd.dma_start(out=xbf, in_=x[b])
        xT = [xq_pool.tile([128, N], BF16, tag=f"xT{t}", name=f"xT{t}", bufs=4)
              for t in range(2)]
        for t in range(2):
            nc.sync.dma_start_transpose(out=xT[t],
                                        in_=xbf[:, t * 128:(t + 1) * 128])
        xT_all.append(xT)

        # ---- conv y[o_t] [128, 320] + bias ----
        ybuf = [mid_pool.tile([128, M], F32, tag=f"ybuf{t}", name=f"ybuf{t}",
                              bufs=4) for t in range(2)]
        for ot in range(2):
            ps = ro_pool.tile([128, 1024], F32, tag="ro", name="ps_conv")
            xs = [xT[t][:, 0:1024].rearrange("p (r a c b) -> p r a c b",
                                             r=16, a=2, c=16, b=2)
                  for t in range(2)]
            first = True
            for cit in range(2):
                for kh in range(2):
                    for kw in range(2):
                        nc.tensor.matmul(
                            ps[:, 0:256],
                            lhsT=srwT[cit][kh * 2 + kw][:, ot * 128:(ot + 1) * 128],
                            rhs=xs[cit][:, :, kh, :, kw],
                            start=first, stop=(cit == 1 and kh == 1 and kw == 1))
                        first = False
            xt_ = [xT[t][:, 1024:1280].rearrange("p (r a c b) -> p r a c b",
                                                 r=8, a=2, c=8, b=2)
                   for t in range(2)]
            first = True
            for cit in range(2):
                for kh in range(2):
                    for kw in range(2):
                        nc.tensor.matmul(
                            ps[:, 512:576],
                            lhsT=srwT[cit][kh * 2 + kw][:, ot * 128:(ot + 1) * 128],
                            rhs=xt_[cit][:, :, kh, :, kw],
                            start=first, stop=(cit == 1 and kh == 1 and kw == 1))
                        first = False
            nc.vector.tensor_scalar_add(ybuf[ot][:, 0:256], ps[:, 0:256],
                                        srb_sb[ot])
            nc.vector.tensor_scalar_add(ybuf[ot][:, 256:320], ps[:, 512:576],
                                        srb_sb[ot])
        ybuf_all.append(ybuf)

        # ---- LN stats ----
        ysq = [mid_pool.tile([128, M], F32, tag=f"ysq{t}", name=f"ysq{t}")
               for t in range(2)]
        for ot in range(2):
            nc.vector.tensor_mul(ysq[ot], ybuf[ot], ybuf[ot])
        ps_stat = ro_pool.tile([128, 1024], F32, tag="ro", name="ps_stat")
        for ot in range(2):
            nc.tensor.matmul(ps_stat[0:1, 0:M], lhsT=ones_col, rhs=ybuf[ot],
                             start=(ot == 0), stop=(ot == 1))
        for ot in range(2):
            nc.tensor.matmul(ps_stat[0:1, 512:512 + M], lhsT=ones_col,
                             rhs=ysq[ot], start=(ot == 0), stop=(ot == 1))
        mu = mid_pool.tile([1, M], F32, tag="mu", name="mu", bufs=4)
        nc.vector.tensor_scalar_mul(mu, ps_stat[0:1, 0:M], 1.0 / C)
        var = mid_pool.tile([1, M], F32, tag="var", name="var")
        musq = mid_pool.tile([1, M], F32, tag="musq", name="musq")
        nc.vector.tensor_mul(musq, mu, mu)
        nc.vector.tensor_scalar_mul(var, ps_stat[0:1, 512:512 + M], 1.0 / C)
        nc.vector.tensor_sub(var, var, musq)
        lnv = mid_pool.tile([1, M], F32, tag="lnv", name="lnv")
        nc.scalar.activation(lnv, var, mybir.ActivationFunctionType.Ln,
                             bias=eps_b, scale=1.0)
        rstd = mid_pool.tile([1, M], F32, tag="rstd", name="rstd", bufs=4)
        nc.scalar.activation(rstd, lnv, mybir.ActivationFunctionType.Exp,
                             bias=zeros_b[0:1], scale=-0.5)
        mu_all.append(mu)
        rstd_all.append(rstd)

    # ------- phase 2: q, y_n, kv, attention (per batch) -------
    for b in range(B_LOC):
        xT = xT_all[b]
        ybuf = ybuf_all[b]
        mu = mu_all[b]
        rstd = rstd_all[b]

        # ---- qT[co_t] = q_w @ x^T  (feature-major) ----
        qT = [xq_pool.tile([128, N], BF16, tag=f"qT{t}", name=f"qT{t}")
              for t in range(2)]
        for cot in range(2):
            for (c0, cw) in CHUNKS:
                ps = ro_pool.tile([128, 1024], F32, tag="ro", name="ps_q")
                for cit in range(2):
                    nc.tensor.matmul(ps[:, 0:cw],
                                     lhsT=qwT[cit][:, cot * 128:(cot + 1) * 128],
                                     rhs=xT[cit][:, c0:c0 + cw],
                                     start=(cit == 0), stop=(cit == 1))
                nc.vector.tensor_copy(qT[cot][:, c0:c0 + cw], ps[:, 0:cw])

        # broadcast mu, r across partitions via K=1 matmul
        ps_bc = ro_pool.tile([128, 1024], F32, tag="ro", name="ps_bc")
        nc.tensor.matmul(ps_bc[:, 0:M], lhsT=ones_row, rhs=mu,
                         start=True, stop=True)
        nc.tensor.matmul(ps_bc[:, 512:512 + M], lhsT=ones_row, rhs=rstd,
                         start=True, stop=True)

        # ---- y_n = (y - mu_b) * (r_b * g) + b   (bf16) ----
        y_n = [mid_pool.tile([128, M], BF16, tag=f"yn{t}", name=f"yn{t}")
               for t in range(2)]
        for ot in range(2):
            t1 = mid_pool.tile([128, M], F32, tag=f"t1_{ot}", name=f"t1_{ot}")
            nc.vector.tensor_sub(t1, ybuf[ot], ps_bc[:, 0:M])
            rg = mid_pool.tile([128, M], F32, tag=f"rg_{ot}", name=f"rg_{ot}")
            nc.vector.tensor_scalar_mul(rg, ps_bc[:, 512:512 + M], g_sb[ot])
            t2 = mid_pool.tile([128, M], F32, tag=f"t2_{ot}", name=f"t2_{ot}")
            nc.vector.tensor_mul(t2, t1, rg)
            nc.vector.tensor_scalar_add(y_n[ot], t2, b_sb[ot])

        # ---- kT (feature-major) and v (token-major) ----
        kT = [mid_pool.tile([128, M], BF16, tag=f"kT{t}", name=f"kT{t}")
              for t in range(2)]
        for kot in range(2):
            ps = ro_pool.tile([128, 1024], F32, tag="ro", name="ps_k")
            for cit in range(2):
                nc.tensor.matmul(ps[:, 0:M],
                                 lhsT=kvwT[cit][:, kot * 128:(kot + 1) * 128],
                                 rhs=y_n[cit],
                                 start=(cit == 0), stop=(cit == 1))
            nc.vector.tensor_copy(kT[kot], ps[:, 0:M])
        v_sb = [mid_pool.tile([128, C], BF16, tag=f"v{i}", name=f"v{i}")
                for i in range(3)]
        for mi, (m0, mw) in enumerate(MTILES):
            ps = ro_pool.tile([128, 1024], F32, tag="ro", name="ps_v")
            for cit in range(2):
                nc.tensor.matmul(ps[0:mw, 0:C],
                                 lhsT=y_n[cit][:, m0:m0 + mw],
                                 rhs=kvwT[cit][:, C:2 * C],
                                 start=(cit == 0), stop=(cit == 1))
            nc.vector.tensor_copy(v_sb[mi][0:mw, :], ps[0:mw, 0:C])
            if mi == 2:
                nc.vector.tensor_copy(v_sb[2][64:128, :], ps[0:64, 0:C])

        # ---- attention ----
        for (c0, cw) in CHUNKS:
            # accumulators: [:, 0:cw] = attnV, [:, 512:512+cw] = colsum
            acc = [ro_pool.tile([128, 1024], F32, tag="ro", name="acc")
                   for _ in range(2)]
            for mi, (m0, mw) in enumerate(MTILES):
                groups = [0, 1] if mi < 2 else [0]
                for g in groups:
                    s_ps = s_pool.tile([128, 4 * 512], F32, tag="s", name="s_ps")
                    for hh in range(4):
                        nc.tensor.matmul(
                            s_ps[0:mw, hh * 512:hh * 512 + cw],
                            lhsT=kT[g][hh * 32:(hh + 1) * 32, m0:m0 + mw],
                            rhs=qT[g][hh * 32:(hh + 1) * 32, c0:c0 + cw],
                            start=True, stop=True,
                            tile_position=(32 * hh, 0))
                    if mi == 2:
                        for hh in range(4):
                            nc.tensor.matmul(
                                s_ps[64:128, hh * 512:hh * 512 + cw],
                                lhsT=kT[1][hh * 32:(hh + 1) * 32, m0:m0 + mw],
                                rhs=qT[1][hh * 32:(hh + 1) * 32, c0:c0 + cw],
                                start=True, stop=True,
                                tile_position=(32 * hh, 64),
                                skip_group_check=True)
                    emw = 128 if mi == 2 else mw
                    p_sb = p_pool.tile([128, 4 * 512], BF16, tag="p", name="p_sb")
                    s_v = s_ps.rearrange("p (h w) -> p h w", h=4)[0:emw, :, 0:cw]
                    p_v = p_sb.rearrange("p (h w) -> p h w", h=4)[0:emw, :, 0:cw]
                    nc.scalar.activation(p_v, s_v,
                                         mybir.ActivationFunctionType.Exp,
                                         bias=zeros_b[0:emw], scale=SCALE)
                    for gg in ([0, 1] if mi == 2 else [g]):
                        rbase = 64 * gg if mi == 2 else 0
                        for hh in range(4):
                            nc.tensor.matmul(
                                acc[gg][32 * hh:32 * (hh + 1), 512:512 + cw],
                                lhsT=ones_bf[rbase:rbase + mw, :],
                                rhs=p_sb[rbase:rbase + mw,
                                         hh * 512:hh * 512 + cw],
                                start=(mi == 0), stop=(mi == 2),
                                tile_position=(rbase, 32 * hh),
                                skip_group_check=(hh > 0))
                            nc.tensor.matmul(
                                acc[gg][32 * hh:32 * (hh + 1), 0:cw],
                                lhsT=v_sb[mi][rbase:rbase + mw,
                                              gg * 128 + hh * 32:
                                              gg * 128 + (hh + 1) * 32],
                                rhs=p_sb[rbase:rbase + mw,
                                         hh * 512:hh * 512 + cw],
                                start=(mi == 0), stop=(mi == 2),
                                tile_position=(rbase, 32 * hh),
                                skip_group_check=(hh > 0))
            outT = []
            for g in range(2):
                rrec = o_pool.tile([128, 512], F32, tag=f"rrec{g}", name=f"rrec{g}")
                nc.vector.reciprocal_approx_fast(rrec[:, 0:cw],
                                                 acc[g][:, 512:512 + cw])
                on = o_pool.tile([128, 512], BF16, tag=f"outT{g}", name=f"outT{g}")
                nc.vector.tensor_mul(on[:, 0:cw], acc[g][:, 0:cw], rrec[:, 0:cw])
                outT.append(on)
            # proj: token-major output tiles
            for nt in range(cw // 128):
                ps = ro_pool.tile([128, 1024], F32, tag="ro", name="ps_proj")
                for ct in range(2):
                    nc.tensor.matmul(ps[:, 0:C],
                                     lhsT=outT[ct][:, nt * 128:(nt + 1) * 128],
                                     rhs=pwT[ct],
                                     start=(ct == 0), stop=(ct == 1))
                ob = o_pool.tile([128, C], F32, tag="ob", name="ob")
                nc.vector.tensor_add(ob, ps[:, 0:C], pb_bc)
                n0 = c0 + nt * 128
                nc.sync.dma_start(out=out[b, n0:n0 + 128, :], in_=ob)


_NC_CACHE = None


def _get_nc():
    global _NC_CACHE
    if _NC_CACHE is None:
        _NC_CACHE = build_kernel()
    return _NC_CACHE


def kernel(**inputs) -> np.ndarray:
    x = np.ascontiguousarray(np.asarray(inputs["x"], dtype=np.float32))
    B = x.shape[0]
    assert x.shape == (32, N, C), x.shape
    weights = {k: np.ascontiguousarray(np.asarray(inputs[k], dtype=np.float32))
               for k in ("q_w", "kv_w", "sr_w", "sr_b", "ln_g", "ln_b",
                         "proj_w", "proj_b")}
    nc = _get_nc()
    in_maps = []
    for core in range(NCORES):
        m = {"x": x[core * B_LOC:(core + 1) * B_LOC]}
        m.update(weights)
        in_maps.append(m)
    res = run_bass_kernel_spmd(nc, in_maps, core_ids=list(range(NCORES)))
    out = np.concatenate([res.results[i]["out"] for i in range(NCORES)], axis=0)
    assert out.shape == (B, N, C)
    return out.astype(np.float32)


if __name__ == "__main__":
    rng = np.random.default_rng(0)
    ins = {
        "x": rng.standard_normal((32, N, C), dtype=np.float32),
        "q_w": rng.standard_normal((C, C), dtype=np.float32) * 0.02,
        "kv_w": rng.standard_normal((2 * C, C), dtype=np.float32) * 0.02,
        "sr_w": rng.standard_normal((C, C, 2, 2), dtype=np.float32) * 0.02,
        "sr_b": np.zeros(C, np.float32),
        "ln_g": np.ones(C, np.float32),
        "ln_b": np.zeros(C, np.float32),
        "proj_w": rng.standard_normal((C, C), dtype=np.float32) * 0.02,
        "proj_b": np.zeros(C, np.float32),
    }
    o = kernel(**ins)
    print("out", o.shape, o.dtype, float(np.abs(o).max()))
